# revision 6
# baseline (speedup 1.0000x reference)
"""All-pairs distance-kernel regressor on 8 Trainium2 NeuronCores.

kernel[i,j] = sigmoid(fc2 @ LeakyReLU(fc1 @ |h_i - h_j| + b1, 0.2) + b2)
for n=1024 nodes, h=128. Returns (hiddens, kernel) like the reference.

Distribution: the pair matrix is symmetric, so each core computes a 640-wide
circulant window of columns for its 128 pivot rows (5 of 8 column blocks;
block distance 0..4). Every (i,j) is covered directly or via the mirrored
(j,i); the host gathers the 8 windows and fills the rest by transposition.
Per-core inputs are pre-rotated so all cores run the same program (SPMD).

Per core (window width W=640):
  S1  (ACT):  D_i = |h_i - hT| as float32r via Abs(scale=-1, bias=h_i)
  T1  (PE) :  z = W1T.T @ D_i -> PSUM [128, W], two pivots stacked
  PRELU     :  r = max(t, 0.2t), t = z + b1 -> SBUF f32r
              ACT pairs: one Prelu op; DVE pairs: tensor_scalar +
              scalar_tensor_tensor, then gpsimd f32->f32r convert
  T2  (PE) :  e rows via [w2;0|0;w2] matmul, 3 pairs per [66, W] PSUM tile
              at base partitions {0,32,64}
  SIG (ACT):  sigmoid(e + b2) over the whole [66, W] tile, strided DMA
              gathers the 6 valid rows per group.
"""

import numpy as np
from contextlib import ExitStack

import concourse.bass as bass
import concourse.tile as tile
from concourse import bacc, mybir
from concourse.bass_utils import run_bass_kernel_spmd

N_NODES = 1024
H_DIM = 128
H_HALF = 64
N_CORES = 8
ROWS = N_NODES // N_CORES        # 128 pivot rows per core
W = 640                          # symmetric column window per core
NEG_SLOPE = 0.2
N_PAIRS = ROWS // 2              # 64
DVE_PRELU = 60                   # pairs whose Prelu runs on DVE (rest on ACT)

F32 = mybir.dt.float32
F32R = mybir.dt.bfloat16  # matmul operand dtype (bf16: offset-writes legal)
AF = mybir.ActivationFunctionType
ALU = mybir.AluOpType

_BUILD_CACHE = {}


def _build():
    nc = bacc.Bacc("TRN2", target_bir_lowering=False, debug=False,
                   num_devices=N_CORES)

    hTw = nc.dram_tensor("hTw", [H_DIM, W], F32, kind="ExternalInput").ap()
    w1t = nc.dram_tensor("w1t", [H_DIM, H_HALF], F32, kind="ExternalInput").ap()
    b1s = nc.dram_tensor("b1s", [H_DIM, 1], F32, kind="ExternalInput").ap()
    w2s = nc.dram_tensor("w2s", [H_DIM, 32], F32, kind="ExternalInput").ap()
    b2c = nc.dram_tensor("b2c", [66, 1], F32, kind="ExternalInput").ap()
    outk = nc.dram_tensor("outk", [ROWS, W], F32, kind="ExternalOutput").ap()

    # pair -> engine for the Prelu stage, spread evenly
    dve_pair = [bool((p * DVE_PRELU) // N_PAIRS
                     != ((p + 1) * DVE_PRELU) // N_PAIRS)
                for p in range(N_PAIRS)]

    with tile.TileContext(nc) as tc, ExitStack() as ctx:
        const = ctx.enter_context(tc.tile_pool(name="const", bufs=1))
        dpool = ctx.enter_context(tc.tile_pool(name="d", bufs=4))
        upool = ctx.enter_context(tc.tile_pool(name="u", bufs=2))
        r32pool = ctx.enter_context(tc.tile_pool(name="r32", bufs=2))
        rpool = ctx.enter_context(tc.tile_pool(name="r", bufs=3))
        spool = ctx.enter_context(tc.tile_pool(name="s", bufs=2))
        zpsum = ctx.enter_context(tc.tile_pool(name="z", bufs=2, space="PSUM"))
        epsum = ctx.enter_context(tc.tile_pool(name="e", bufs=2, space="PSUM"))

        hTw_t = const.tile([H_DIM, W], F32)
        nc.sync.dma_start(hTw_t[:], hTw)
        w1t_t = const.tile([H_DIM, H_HALF], F32)
        nc.sync.dma_start(w1t_t[:], w1t)
        b1s_t = const.tile([H_DIM, 1], F32)
        nc.sync.dma_start(b1s_t[:], b1s)
        w2s_t = const.tile([H_DIM, 32], F32)
        nc.sync.dma_start(w2s_t[:], w2s)
        b2c_t = const.tile([66, 1], F32)
        nc.sync.dma_start(b2c_t[:], b2c)

        w1t_r = const.tile([H_DIM, H_HALF], F32R)
        nc.vector.tensor_copy(w1t_r[:], w1t_t[:])
        w2s_r = const.tile([H_DIM, 32], F32R)
        nc.vector.tensor_copy(w2s_r[:], w2s_t[:])

        e_ps = None
        for p in range(N_PAIRS):
            g, slot = divmod(p, 3)
            if slot == 0:
                e_ps = epsum.tile([96, W], F32, tag="e")
            z_ps = zpsum.tile([128, W], F32, tag="z")
            for ab in range(2):
                i = 2 * p + ab
                d_t = dpool.tile([H_DIM, W], F32R, tag="d")
                # D = |h_i - hT| (f32r), Abs(-hTw + h_i)
                nc.scalar.activation(d_t[:], hTw_t[:], AF.Abs,
                                     bias=hTw_t[:, i:i + 1], scale=-1.0)
                nc.tensor.matmul(z_ps[ab * H_HALF:(ab + 1) * H_HALF, 0:512],
                                 w1t_r[:], d_t[:, 0:512],
                                 start=True, stop=True)
                nc.tensor.matmul(z_ps[ab * H_HALF:(ab + 1) * H_HALF, 512:W],
                                 w1t_r[:], d_t[:, 512:W],
                                 start=True, stop=True)
            r_t = rpool.tile([128, W], F32R, tag="r")
            if dve_pair[p]:
                u_t = upool.tile([128, W], F32, tag="u")
                nc.vector.tensor_scalar(u_t[:], z_ps[:], b1s_t[:, 0:1],
                                        NEG_SLOPE, op0=ALU.add, op1=ALU.mult)
                r32_t = r32pool.tile([128, W], F32, tag="r32")
                nc.vector.scalar_tensor_tensor(r32_t[:], z_ps[:],
                                               b1s_t[:, 0:1], u_t[:],
                                               op0=ALU.add, op1=ALU.max)
                nc.gpsimd.tensor_copy(r_t[:], r32_t[:])
            else:
                nc.scalar.activation(r_t[:], z_ps[:], AF.Prelu,
                                     bias=b1s_t[:, 0:1], scale=1.0,
                                     alpha=NEG_SLOPE)
            o = slot * 32
            nc.tensor.matmul(e_ps[o:o + 32, 0:512], w2s_r[:], r_t[:, 0:512],
                             start=True, stop=True)
            nc.tensor.matmul(e_ps[o:o + 32, 512:W], w2s_r[:], r_t[:, 512:W],
                             start=True, stop=True)
            if slot == 2 or p == N_PAIRS - 1:
                rows = 2 * (slot + 1)
                s_t = spool.tile([96, W], F32, tag="s")
                nc.scalar.activation(s_t[0:o + 2, :], e_ps[0:o + 2, :],
                                     AF.Sigmoid, bias=b2c_t[0:o + 2, 0:1],
                                     scale=1.0)
                for sidx in range(slot + 1):
                    nc.sync.dma_start(
                        outk[6 * g + 2 * sidx:6 * g + 2 * sidx + 2, :],
                        s_t[32 * sidx:32 * sidx + 2, :])

    nc.compile()
    return nc


def kernel(hiddens, fc1_w, fc1_b, fc2_w, fc2_b):
    hiddens = np.asarray(hiddens, dtype=np.float32)
    fc1_w = np.asarray(fc1_w, dtype=np.float32)
    fc1_b = np.asarray(fc1_b, dtype=np.float32)
    fc2_w = np.asarray(fc2_w, dtype=np.float32)
    fc2_b = np.asarray(fc2_b, dtype=np.float32)

    if "nc" not in _BUILD_CACHE:
        _BUILD_CACHE["nc"] = _build()
    nc = _BUILD_CACHE["nc"]

    hT = np.ascontiguousarray(hiddens.T)                    # [128, 1024]
    w1t = np.ascontiguousarray(fc1_w.T)                     # [128, 64]
    b1s = np.concatenate([fc1_b, fc1_b]).reshape(H_DIM, 1)
    w2s = np.zeros((H_DIM, 32), dtype=np.float32)
    w2s[:H_HALF, 0] = fc2_w[0]
    w2s[H_HALF:, 1] = fc2_w[0]
    b2c = np.full((66, 1), fc2_b[0], dtype=np.float32)

    in_maps = []
    for c in range(N_CORES):
        hTrot = np.roll(hT, -ROWS * c, axis=1)
        in_maps.append({
            "hTw": np.ascontiguousarray(hTrot[:, :W]),
            "w1t": w1t, "b1s": b1s, "w2s": w2s, "b2c": b2c,
        })

    res = run_bass_kernel_spmd(nc, in_maps, core_ids=list(range(N_CORES)))

    full = np.full((N_NODES, N_NODES), np.nan, dtype=np.float32)
    for c in range(N_CORES):
        cols = (np.arange(W) + ROWS * c) % N_NODES
        full[ROWS * c:ROWS * (c + 1), cols] = res.results[c]["outk"]
    mask = np.isnan(full)
    full[mask] = full.T[mask]
    assert not np.isnan(full).any()
    return hiddens, full
